# revision 1
# baseline (speedup 1.0000x reference)
"""EngramEmbeddings Trainium2 kernel.

Expert-sharded across 8 NeuronCores: core c owns head c of the n=2 and n=3
hash tables and computes the hashed-ngram embedding lookup for all
B*S = 32768 tokens for its two slots.

Device-side work per core:
  1. int64 hash (id*seed per ngram term, XOR, mod table_size) computed
     exactly with 16-bit limb arithmetic on the vector engine (DVE
     arithmetic is fp32 internally, so every arithmetic intermediate stays
     < 2^24; bit surgery uses exact int32 bitwise/shift ops; mod is
     reciprocal-multiply + floor + one conditional subtract, exact).
  2. n2 slot (table < 32768 rows): dma_gather (fast Q7 ucode, int16
     indices, 512B-padded rows) — 8 instructions of 4096 rows each.
     Tokens are host-permuted into the ucode's wrapped stream order so
     gathered rows land p-major for contiguous stores.
  3. n3 slot (table > 32768 rows, exceeds dma_gather's int16 reach):
     indirect DMA, one 128-row instruction per token column.

Host does sharding-style prep only: dtype casts, per-batch-row
shift/padding of ids, token-order permutations, splitting the runtime
seeds into 16-bit halves, table concat/pad, and final output stitching.
"""

import numpy as np

try:
    import concourse  # noqa: F401
except ImportError:  # pragma: no cover
    import sys

    for _p in ("/opt/trn_rl_repo", "/root/.axon_site/_ro/trn_rl_repo"):
        if _p not in sys.path:
            sys.path.insert(0, _p)

import concourse.bass as bass
import concourse.tile as tile
from concourse import bacc, mybir
from concourse.bass_utils import run_bass_kernel_spmd

N2_SIZES = [6619, 6637, 6653, 6659, 6661, 6673, 6679, 6689]
N3_SIZES = [65521, 65537, 65539, 65543, 65551, 65557, 65563, 65579]
B, S = 8, 4096
P = 128
NTOK = B * S              # 32768
TPB = NTOK // P           # 256 tokens per partition (p-major: token = p*256+t)
SLOT = 80
V2 = max(N2_SIZES)        # 6689
V3 = max(N3_SIZES)        # 65579
E2 = 128                  # n2 table row padded to 128 f32 = 512B for dma_gather
NCH = 8                   # n2 dma_gather chunks (4096 tokens each)
CW = TPB // NCH           # 32 token-columns per chunk
N3C = 4                   # n3 hash computed in 4 slabs of 64 columns
TT3 = 32                  # n3 gather columns per store tile

_NC = None
TRACE = False
LAST_RESULT = None

# token permutation for the n2 dma_gather stream: chunk a's stream position
# j = c*16 + q (ucode wrapped order: lane q = j%16, col c = j//16) gathers
# the token landing at dest (partition u = j%128, block b = j//128), which
# we choose to be p-major token u*256 + 32a + b.  Hash lane (pi = 16a+q, c)
# therefore holds token TAU2[pi, c].
_c = np.arange(TPB)[None, :]
_pi = np.arange(P)[:, None]
TAU2 = ((16 * (_c % 8) + _pi % 16) * 256 + 32 * (_pi // 16) + _c // 8).astype(
    np.int64
)


def _build_nc():
    dt = mybir.dt
    A = mybir.AluOpType
    AND, XOR = A.bitwise_and, A.bitwise_xor
    LSR, LSL = A.logical_shift_right, A.logical_shift_left
    ADD, MULT, SUB, GE = A.add, A.mult, A.subtract, A.is_ge
    i32 = dt.int32
    f32 = dt.float32

    nc = bacc.Bacc("TRN2", target_bir_lowering=False, debug=False,
                   num_swdge_queues=4)
    tbl3 = nc.dram_tensor("tbl3", [V3, SLOT], f32, kind="ExternalInput")
    tbl2 = nc.dram_tensor("tbl2", [V2, E2], f32, kind="ExternalInput")
    ids3d = nc.dram_tensor("ids3", [3, NTOK], i32, kind="ExternalInput")
    ids2d = nc.dram_tensor("ids2", [2, NTOK], i32, kind="ExternalInput")
    s03d = nc.dram_tensor("s0w3", [P, 3 * TPB], i32, kind="ExternalInput")
    s13d = nc.dram_tensor("s1w3", [P, 3 * TPB], i32, kind="ExternalInput")
    s02d = nc.dram_tensor("s0w2", [P, 2 * TPB], i32, kind="ExternalInput")
    s12d = nc.dram_tensor("s1w2", [P, 2 * TPB], i32, kind="ExternalInput")
    # per-slot mod constants: M, R16, R24, R32, R40 (int32) + inv (f32)
    cst3d = nc.dram_tensor("cst3", [P, 5 * TPB], i32, kind="ExternalInput")
    cst2d = nc.dram_tensor("cst2", [P, 5 * TPB], i32, kind="ExternalInput")
    inv3d = nc.dram_tensor("inv3", [P, TPB], f32, kind="ExternalInput")
    inv2d = nc.dram_tensor("inv2", [P, TPB], f32, kind="ExternalInput")
    out3d = nc.dram_tensor("out3", [NTOK, SLOT], f32, kind="ExternalOutput")
    out2d = nc.dram_tensor("out2", [NTOK, E2], f32, kind="ExternalOutput")

    with tile.TileContext(nc) as tc:
        with (
            tc.tile_pool(name="c", bufs=1) as cp,
            tc.tile_pool(name="w", bufs=1) as wp,
            tc.tile_pool(name="g", bufs=1) as gp,
        ):

            def ld(dram, shape, dtype, tag):
                t = cp.tile(shape, dtype, tag=tag, name=tag)
                nc.sync.dma_start(t[:], dram.ap())
                return t

            s0w3 = ld(s03d, [P, 3 * TPB], i32, "s0w3")
            s1w3 = ld(s13d, [P, 3 * TPB], i32, "s1w3")
            s0w2 = ld(s02d, [P, 2 * TPB], i32, "s0w2")
            s1w2 = ld(s12d, [P, 2 * TPB], i32, "s1w2")
            cst3 = ld(cst3d, [P, 5 * TPB], i32, "cst3")
            cst2 = ld(cst2d, [P, 5 * TPB], i32, "cst2")
            inv3 = ld(inv3d, [P, TPB], f32, "inv3")
            inv2 = ld(inv2d, [P, TPB], f32, "inv2")

            ids3v = ids3d.ap().rearrange("r (p t) -> r p t", p=P)
            ids2v = ids2d.ap().rearrange("r (p t) -> r p t", p=P)
            id3 = []
            for r in range(3):
                t_ = cp.tile([P, TPB], i32, tag=f"id3_{r}", name=f"id3_{r}")
                nc.sync.dma_start(t_[:], ids3v[r])
                id3.append(t_)
            pv2_3, prv_3, cur_3 = id3  # rows: 0=pv2, 1=prv, 2=cur
            id2 = []
            for r in range(2):
                t_ = cp.tile([P, TPB], i32, tag=f"id2_{r}", name=f"id2_{r}")
                nc.sync.dma_start(t_[:], ids2v[r])
                id2.append(t_)
            prv_2, cur_2 = id2  # rows: 0=prv, 1=cur

            def hash_idx(srcs, s0w, s1w, cst, inv, C, col0, tagp):
                """Hashed table index for one slot over C token columns.

                srcs: list of id tiles (one per ngram term, cols col0..+C
                used).  All wide ops on [P, npair*C]; returns int32 idx tile
                [P, C] with values in [0, m).
                """
                npair = len(srcs)
                W = npair * C

                def wt():
                    return wp.tile([P, W], i32, tag=f"w{tagp}", bufs=8,
                                   name=f"w{tagp}_{nc.next_id()}")

                def st(dtype=i32):
                    return wp.tile([P, C], dtype, tag=f"s{tagp}{dtype}",
                                   bufs=10, name=f"s{tagp}_{nc.next_id()}")

                sl = [slice(j * C, (j + 1) * C) for j in range(npair)]
                cs = slice(col0, col0 + C)
                Mt = cst[:, 0 * TPB + col0 : 0 * TPB + col0 + C]
                R16 = cst[:, 1 * TPB + col0 : 1 * TPB + col0 + C]
                R24 = cst[:, 2 * TPB + col0 : 2 * TPB + col0 + C]
                R32 = cst[:, 3 * TPB + col0 : 3 * TPB + col0 + C]
                R40 = cst[:, 4 * TPB + col0 : 4 * TPB + col0 + C]
                INV = inv[:, cs]
                s0v = [s0w[:, j * TPB + col0 : j * TPB + col0 + C]
                       for j in range(npair)]
                s1v = [s1w[:, j * TPB + col0 : j * TPB + col0 + C]
                       for j in range(npair)]

                X = wt()
                for j, src in enumerate(srcs):
                    nc.scalar.copy(X[:, sl[j]], src[:, cs])
                a0 = wt()
                nc.vector.tensor_scalar(a0[:], X[:], 0xFF, None, AND)
                a1 = wt()
                nc.vector.tensor_scalar(a1[:], X[:], 8, None, LSR)
                t00, t10, t01, t11 = wt(), wt(), wt(), wt()
                for tt_, aa, ssv in ((t00, a0, s0v), (t10, a1, s0v),
                                     (t01, a0, s1v), (t11, a1, s1v)):
                    for j in range(npair):
                        nc.vector.tensor_tensor(tt_[:, sl[j]], aa[:, sl[j]],
                                                ssv[j], MULT)
                Apt = wt()
                nc.vector.tensor_scalar(Apt[:], t10[:], 0xFF, 8, AND, LSL)
                v0a = wt()
                nc.vector.tensor_scalar(v0a[:], t00[:], 0xFFFF, None, AND)
                v0 = wt()
                nc.vector.tensor_tensor(v0[:], v0a[:], Apt[:], ADD)
                L0 = wt()
                nc.vector.tensor_scalar(L0[:], v0[:], 0xFFFF, None, AND)
                c0 = wt()
                nc.vector.tensor_scalar(c0[:], v0[:], 16, None, LSR)
                u1a = wt()
                nc.vector.tensor_scalar(u1a[:], t10[:], 8, None, LSR)
                u1 = wt()
                nc.vector.tensor_tensor(u1[:], u1a[:], c0[:], ADD)
                u2a = wt()
                nc.vector.tensor_scalar(u2a[:], t01[:], 0xFFFF, None, AND)
                u2 = wt()
                nc.vector.tensor_tensor(u2[:], u2a[:], u1[:], ADD)
                u3a = wt()
                nc.vector.tensor_scalar(u3a[:], t00[:], 16, None, LSR)
                v1 = wt()
                nc.vector.tensor_tensor(v1[:], u3a[:], u2[:], ADD)
                Ff = wt()
                nc.vector.tensor_scalar(Ff[:], t11[:], 0xFF, 8, AND, LSL)
                v1b = wt()
                nc.vector.tensor_tensor(v1b[:], v1[:], Ff[:], ADD)
                L1 = wt()
                nc.vector.tensor_scalar(L1[:], v1b[:], 0xFFFF, None, AND)
                c1 = wt()
                nc.vector.tensor_scalar(c1[:], v1b[:], 16, None, LSR)
                v2a = wt()
                nc.vector.tensor_scalar(v2a[:], t01[:], 16, None, LSR)
                v2 = wt()
                nc.vector.tensor_tensor(v2[:], v2a[:], c1[:], ADD)
                L2a = wt()
                nc.vector.tensor_scalar(L2a[:], t11[:], 8, None, LSR)
                L2 = wt()
                nc.vector.tensor_tensor(L2[:], L2a[:], v2[:], ADD)

                # xor across pairs -> H limbs [P, C]
                H = []
                for Lt in (L0, L1, L2):
                    Ht = st()
                    nc.vector.tensor_tensor(Ht[:], Lt[:, sl[0]], Lt[:, sl[1]],
                                            XOR)
                    for j in range(2, npair):
                        nc.vector.tensor_tensor(Ht[:], Ht[:], Lt[:, sl[j]],
                                                XOR)
                    H.append(Ht)
                H0, H1, H2 = H

                def mod_m(x):
                    y = st(f32)
                    nc.vector.tensor_tensor(y[:], x[:], INV, MULT)
                    y2 = st(f32)
                    nc.vector.tensor_scalar(y2[:], y[:], 0.5, None, SUB)
                    q = st()
                    nc.vector.tensor_copy(q[:], y2[:])
                    qm = st()
                    nc.vector.tensor_tensor(qm[:], q[:], Mt, MULT)
                    r = st()
                    nc.vector.tensor_tensor(r[:], x[:], qm[:], SUB)
                    ge = st()
                    nc.vector.tensor_tensor(ge[:], r[:], Mt, GE)
                    gm = st()
                    nc.vector.tensor_tensor(gm[:], ge[:], Mt, MULT)
                    r2 = st()
                    nc.vector.tensor_tensor(r2[:], r[:], gm[:], SUB)
                    return r2

                H1a = st()
                nc.vector.tensor_scalar(H1a[:], H1[:], 0xFF, None, AND)
                H1b = st()
                nc.vector.tensor_scalar(H1b[:], H1[:], 8, None, LSR)
                H2a = st()
                nc.vector.tensor_scalar(H2a[:], H2[:], 0xFF, None, AND)
                H2b = st()
                nc.vector.tensor_scalar(H2b[:], H2[:], 8, None, LSR)
                ps = []
                for piece, R in ((H1a, R16), (H1b, R24), (H2a, R32),
                                 (H2b, R40)):
                    pp = st()
                    nc.vector.tensor_tensor(pp[:], piece[:], R, MULT)
                    ps.append(mod_m(pp))
                x1 = st()
                nc.vector.tensor_tensor(x1[:], H0[:], ps[0][:], ADD)
                x2 = st()
                nc.vector.tensor_tensor(x2[:], ps[1][:], ps[2][:], ADD)
                x3 = st()
                nc.vector.tensor_tensor(x3[:], x1[:], x2[:], ADD)
                x4 = st()
                nc.vector.tensor_tensor(x4[:], x3[:], ps[3][:], ADD)
                return mod_m(x4)

            idx2_16 = cp.tile([P, TPB], dt.int16, tag="idx2_16", name="idx2_16")
            idx3 = cp.tile([P, TPB], i32, tag="idx3", name="idx3")
            C3 = TPB // N3C
            out2v = out2d.ap().rearrange("(p t) d -> p t d", p=P)
            out3v = out3d.ap().rearrange("(p t) d -> p t d", p=P)

            def n3_slab(c0_, cw_):
                r = hash_idx([pv2_3, prv_3, cur_3], s0w3, s1w3, cst3, inv3,
                             cw_, c0_, "n3")
                nc.vector.tensor_copy(idx3[:, c0_ : c0_ + cw_], r[:])

            def n3_gather_tile(it):
                d3 = gp.tile([P, TT3 * SLOT], f32, tag="d3", bufs=3,
                             name=f"d3_{it}")
                for tt in range(TT3):
                    col = it * TT3 + tt
                    nc.gpsimd.indirect_dma_start(
                        out=d3[:, tt * SLOT : (tt + 1) * SLOT],
                        out_offset=None,
                        in_=tbl3.ap(),
                        in_offset=bass.IndirectOffsetOnAxis(
                            ap=idx3[:, col : col + 1], axis=0
                        ),
                    )
                nc.sync.dma_start(
                    out3v[:, it * TT3 : (it + 1) * TT3, :],
                    d3[:].rearrange("p (t d) -> p t d", d=SLOT),
                )

            def n2_chunk(a):
                # queue q's tx core reads indices from partition group 2q+1
                q = 1 + a % 3
                stg = gp.tile([P, TPB], dt.int16, tag=f"stg{q}", bufs=2,
                              name=f"stg{a}")
                nc.sync.dma_start(stg[32 * q : 32 * q + 16, :],
                                  idx2_16[16 * a : 16 * a + 16, :])
                nc.sync.dma_start(stg[32 * q + 16 : 32 * q + 32, :],
                                  idx2_16[16 * a : 16 * a + 16, :])
                d2 = gp.tile([P, CW * E2], f32, tag="d2", bufs=4,
                             name=f"d2_{a}")
                nc.gpsimd.dma_gather(
                    d2[:].rearrange("p (b e) -> p b e", e=E2),
                    tbl2.ap(),
                    stg[:],
                    4096,
                    4096,
                    E2,
                    single_packet=False,
                    queue_num=q,
                )
                nc.sync.dma_start(
                    out2v[:, CW * a : CW * (a + 1), :],
                    d2[:].rearrange("p (b e) -> p b e", e=E2),
                )

            # small first slab so the indirect stream starts ASAP
            slabs = [(0, 32), (32, 32), (64, 64), (128, 64), (192, 64)]
            done = 0

            def emit_slab(c0_, cw_):
                n3_slab(c0_, cw_)
                for it in range(c0_ // TT3, (c0_ + cw_) // TT3):
                    n3_gather_tile(it)

            emit_slab(*slabs[0])
            emit_slab(*slabs[1])
            # n2 hash + all chunks back-to-back so the dma_gathers pipeline
            # across their three queues
            idx2 = hash_idx([prv_2, cur_2], s0w2, s1w2, cst2, inv2,
                            TPB, 0, "n2")
            nc.vector.tensor_copy(idx2_16[:], idx2[:])
            for a in range(NCH):
                n2_chunk(a)
            for sl_ in slabs[2:]:
                emit_slab(*sl_)

    nc.compile()
    return nc


def _get_nc():
    global _NC
    if _NC is None:
        _NC = _build_nc()
    return _NC


def _broadcast_rows(row):
    return np.ascontiguousarray(np.broadcast_to(row, (P, row.shape[0])))


def _mod_consts(m):
    return [m, 2**16 % m, 2**24 % m, 2**32 % m, 2**40 % m]


def _make_in_maps(inputs):
    ids = np.asarray(inputs["canonical_ids"]).astype(np.int32)  # [B, S]
    hs = np.asarray(inputs["hash_seeds"]).astype(np.int64)      # [3, 8]
    cur = ids.reshape(-1)
    prv = np.pad(ids, ((0, 0), (1, 0)))[:, :S].reshape(-1)
    pv2 = np.pad(ids, ((0, 0), (2, 0)))[:, :S].reshape(-1)
    ids3 = np.ascontiguousarray(np.stack([pv2, prv, cur]).astype(np.int32))
    ids2 = np.ascontiguousarray(
        np.stack([prv[TAU2], cur[TAU2]]).reshape(2, NTOK).astype(np.int32)
    )

    maps = []
    for c in range(8):
        s0, s1, s2 = int(hs[0, c]), int(hs[1, c]), int(hs[2, c])
        m2, m3 = N2_SIZES[c], N3_SIZES[c]

        def seed_row(seeds, lo):
            return np.concatenate(
                [np.full(TPB, (sd & 0xFFFF) if lo else (sd >> 16), np.int32)
                 for sd in seeds]
            )

        cst3row = np.concatenate(
            [np.full(TPB, v, np.int32) for v in _mod_consts(m3)]
        )
        cst2row = np.concatenate(
            [np.full(TPB, v, np.int32) for v in _mod_consts(m2)]
        )
        inv3row = np.full(TPB, np.float64(1.0 / m3) * (1 - 1e-6), np.float32)
        inv2row = np.full(TPB, np.float64(1.0 / m2) * (1 - 1e-6), np.float32)

        tbl3 = np.zeros((V3, SLOT), np.float32)
        w3 = np.asarray(inputs[f"w_n3_h{c}"], dtype=np.float32)
        tbl3[: w3.shape[0]] = w3
        tbl2 = np.zeros((V2, E2), np.float32)
        w2 = np.asarray(inputs[f"w_n2_h{c}"], dtype=np.float32)
        tbl2[: w2.shape[0], :SLOT] = w2

        maps.append(
            {
                "tbl3": tbl3,
                "tbl2": tbl2,
                "ids3": ids3,
                "ids2": ids2,
                "s0w3": _broadcast_rows(seed_row([s0, s1, s2], True)),
                "s1w3": _broadcast_rows(seed_row([s0, s1, s2], False)),
                "s0w2": _broadcast_rows(seed_row([s0, s1], True)),
                "s1w2": _broadcast_rows(seed_row([s0, s1], False)),
                "cst3": _broadcast_rows(cst3row),
                "cst2": _broadcast_rows(cst2row),
                "inv3": _broadcast_rows(inv3row),
                "inv2": _broadcast_rows(inv2row),
            }
        )
    return maps


def kernel(**inputs):
    global LAST_RESULT
    nc = _get_nc()
    in_maps = _make_in_maps(inputs)
    res = run_bass_kernel_spmd(nc, in_maps, core_ids=list(range(8)),
                               trace=TRACE)
    LAST_RESULT = res
    out = np.empty((B, S, 16 * SLOT), np.float32)
    for c in range(8):
        o2 = res.results[c]["out2"].reshape(B, S, E2)
        o3 = res.results[c]["out3"].reshape(B, S, SLOT)
        out[:, :, c * SLOT : (c + 1) * SLOT] = o2[:, :, :SLOT]
        out[:, :, (8 + c) * SLOT : (9 + c) * SLOT] = o3
    return out

